# revision 3
# baseline (speedup 1.0000x reference)
"""CenterLoss on 8 Trainium2 NeuronCores.

loss = sum(clip(distmat * onehot_mask, 1e-12, 1e12)) / B
     = ( sum_b clip(||x_b - centers[labels_b]||^2, 1e-12, 1e12)
         + (B*C - B) * 1e-12 ) / B

Data-parallel over batch: each of the 8 cores gets 512 samples, gathers its
512 center rows from a replicated centers table with an indirect DMA, and
reduces to one partial scalar. Host sums the 8 partials.
"""

import sys

if "/opt/trn_rl_repo" not in sys.path:
    sys.path.insert(0, "/opt/trn_rl_repo")

import numpy as np

B = 4096
D = 512
C = 10000
N_CORES = 8
BL = B // N_CORES  # 512 samples per core
P = 128
NT = BL // P  # 4 tiles of 128 samples
CLAMP_MIN = 1e-12
CLAMP_MAX = 1e12

_nc_cache = None


def _build():
    from concourse import bacc, mybir
    from concourse.bass import IndirectOffsetOnAxis
    from concourse.tile import TileContext

    f32 = mybir.dt.float32
    i32 = mybir.dt.int32

    nc = bacc.Bacc("TRN2", target_bir_lowering=False, debug=False)
    x = nc.dram_tensor("x", [BL, D], f32, kind="ExternalInput")
    labels = nc.dram_tensor("labels", [P, NT], i32, kind="ExternalInput")
    centers = nc.dram_tensor("centers", [C, D], f32, kind="ExternalInput")
    out = nc.dram_tensor("out", [1, 1], f32, kind="ExternalOutput")

    with TileContext(nc) as tc:
        with (
            tc.tile_pool(name="io", bufs=NT) as io,
            tc.tile_pool(name="acc", bufs=1) as accp,
            tc.tile_pool(name="psum", bufs=1, space="PSUM") as psp,
        ):
            ones = accp.tile([P, 1], f32)
            nc.vector.memset(ones[:], 1.0)

            lab = accp.tile([P, NT], i32)
            nc.sync.dma_start(out=lab[:], in_=labels[:, :])

            dsum = accp.tile([P, NT], f32)
            for t in range(NT):
                xt = io.tile([P, D], f32, tag="xt")
                ct = io.tile([P, D], f32, tag="ct")
                nc.sync.dma_start(out=xt[:], in_=x[t * P : (t + 1) * P, :])
                nc.gpsimd.indirect_dma_start(
                    out=ct[:],
                    out_offset=None,
                    in_=centers[:],
                    in_offset=IndirectOffsetOnAxis(ap=lab[:, t : t + 1], axis=0),
                )
                diff = io.tile([P, D], f32, tag="diff")
                nc.vector.tensor_tensor(
                    out=diff[:], in0=xt[:], in1=ct[:], op=mybir.AluOpType.subtract
                )
                sq = io.tile([P, D], f32, tag="sq")
                nc.scalar.activation(
                    out=sq[:],
                    in_=diff[:],
                    func=mybir.ActivationFunctionType.Square,
                    accum_out=dsum[:, t : t + 1],
                )

            clipped = accp.tile([P, NT], f32)
            nc.vector.tensor_scalar(
                out=clipped[:],
                in0=dsum[:],
                scalar1=CLAMP_MIN,
                scalar2=CLAMP_MAX,
                op0=mybir.AluOpType.max,
                op1=mybir.AluOpType.min,
            )
            ps = psp.tile([1, NT], f32)
            nc.tensor.matmul(
                out=ps[:], lhsT=ones[:], rhs=clipped[:], start=True, stop=True
            )
            res = accp.tile([1, 1], f32)
            nc.vector.tensor_reduce(
                out=res[:],
                in_=ps[:],
                axis=mybir.AxisListType.X,
                op=mybir.AluOpType.add,
            )
            nc.sync.dma_start(out=out[:, :], in_=res[:])

    nc.finalize()
    return nc


def _get_nc():
    global _nc_cache
    if _nc_cache is None:
        _nc_cache = _build()
    return _nc_cache


def run_spmd(x, labels, centers, **spmd_kwargs):
    """Shard, run the bass kernel on 8 cores, return (partials, BassKernelResults)."""
    from concourse.bass_utils import run_bass_kernel_spmd

    x = np.ascontiguousarray(np.asarray(x, dtype=np.float32))
    centers = np.ascontiguousarray(np.asarray(centers, dtype=np.float32))
    lab = np.asarray(labels).astype(np.int32).reshape(B)

    in_maps = []
    for k in range(N_CORES):
        lab_k = lab[k * BL : (k + 1) * BL].reshape(NT, P).T  # [P, NT]
        in_maps.append(
            {
                "x": x[k * BL : (k + 1) * BL],
                "labels": np.ascontiguousarray(lab_k),
                "centers": centers,
            }
        )

    res = run_bass_kernel_spmd(
        _get_nc(), in_maps, core_ids=list(range(N_CORES)), **spmd_kwargs
    )
    partials = np.array(
        [res.results[k]["out"][0, 0] for k in range(N_CORES)], dtype=np.float64
    )
    return partials, res


def kernel(x, labels, centers):
    partials, _ = run_spmd(x, labels, centers)
    total = partials.sum() + (B * C - B) * CLAMP_MIN
    return np.asarray(total / B, dtype=np.float32)
